# revision 1
# baseline (speedup 1.0000x reference)
"""Trainium2 Bass kernel: fused attention block (QKV proj -> MHA -> out proj).

Reference (per batch item b, NUM_HEADS=12, Dh=64):
    qkv = x @ W_qkv; q,k,v per head
    attn = softmax(q @ k^T / 8) @ v
    out  = concat_heads(attn) @ W_proj + b_proj

Sharding: data-parallel over batch across 8 NeuronCores (128 batch items
per core), weights replicated. One SPMD Bass program, per-core inputs.

Per-core plan (128 batches, groups of G=8 batches = 392 tokens).
All matmuls use float32r (full PE rate, ~1.6e-4 rel err; requires even
moving dim N and even psum column offsets -> 50-wide per-batch slots).

  A. DMA x token-major, PE-transpose to feature-major xT
  B. q,k GEMM feature-major: psum[128co, T] = Wqkv_tile.T @ xT.
     q co-tiles stored naturally [128, T+2]; k co-tiles scattered into
     block-diagonal form kbd[j]: per batch a [128, 98] block with
     k_h(2j) in rows 0:64 cols 0:49 and k_h(2j+1) in rows 64:128
     cols 49:98 (zeros elsewhere, pre-loaded once from a host constant).
  C. v GEMM token-major -> scratch; SBUF->SBUF DMA scatter into vbd[j]:
     per batch a [98, 128] block with v_h(2j) rows 0:49 cols 0:64 and
     v_h(2j+1) rows 49:98 cols 64:128 (zeros preloaded once).
  D. attention per (head-pair j, batch): both heads in one matmul chain:
       sT2 = kbd_b.T @ q_pair          [98, 50]  scores, heads stacked
       eT2 = exp(sT2/8)                ACT, one op per pair-cell [98,400]
       r2  = onesbd.T @ eT2            [2, 400]  row sums per head
       po  = vbd_b.T @ eT2             [128, 50] unnorm out^T, both heads
     r rows gathered (ACT copy + SBUF-shift DMA) into rgrp[12, T];
     one batched reciprocal per group; per j: broadcast matmul
     (sel2 selector) -> bc[128, T]; unT[j] *= bc (in-place DVE).
  E. proj GEMM token-major: psum[tok, 384] = unT_toktile.T @ Wproj + bias
     -> DMA out (contiguous rows)
"""
import sys

sys.path.insert(0, "/opt/trn_rl_repo")

import numpy as np

NUM_CORES = 8
B_CORE = 128          # batch items per core
SEQ = 49              # tokens per batch item
C = 768               # channels
H = 12                # heads
G = 8                 # batch items per group
T = SEQ * G           # 392 tokens per group (even)
TP = T + 2            # padded q tile width
TOK = B_CORE * SEQ    # 6272 tokens per core
N_GROUPS = B_CORE // G
KBD_W = G * 98        # kbd block row width per j
VBD_W = G * 128       # vbd block row width per j

_CACHE = {}


def _consts():
    ones1 = np.ones((1, 128), dtype=np.float32)
    ident = np.eye(128, dtype=np.float32)
    sel2 = np.zeros((H, 6 * 128), dtype=np.float32)
    for j in range(6):
        sel2[2 * j, 128 * j:128 * j + 64] = 1.0
        sel2[2 * j + 1, 128 * j + 64:128 * (j + 1)] = 1.0
    onesbd = np.zeros((98, 2), dtype=np.float32)
    onesbd[0:49, 0] = 1.0
    onesbd[49:98, 1] = 1.0
    kbdz = np.zeros((128, 6 * KBD_W), dtype=np.float32)
    vbdz = np.zeros((98, 6 * VBD_W), dtype=np.float32)
    return {"ones1": ones1, "ident": ident, "sel2": sel2,
            "onesbd": onesbd, "kbdz": kbdz, "vbdz": vbdz}


def _build():
    import concourse.bacc as bacc
    import concourse.mybir as mybir
    import concourse.tile as tile

    F32 = mybir.dt.float32
    F32R = mybir.dt.float32r
    EXP = mybir.ActivationFunctionType.Exp

    nc = bacc.Bacc("TRN2", target_bir_lowering=False)

    d_x = nc.declare_dram_parameter("x", [TOK, C], F32, isOutput=False)
    d_wqkv = nc.declare_dram_parameter("wqkv", [C, 3 * C], F32R, isOutput=False)
    d_wproj = nc.declare_dram_parameter("wproj", [C, C], F32R, isOutput=False)
    d_bias = nc.declare_dram_parameter("bias", [1, C], F32R, isOutput=False)
    d_ones1 = nc.declare_dram_parameter("ones1", [1, 128], F32R, isOutput=False)
    d_ident = nc.declare_dram_parameter("ident", [128, 128], F32, isOutput=False)
    d_sel2 = nc.declare_dram_parameter("sel2", [H, 6 * 128], F32R, isOutput=False)
    d_onesbd = nc.declare_dram_parameter("onesbd", [98, 2], F32R, isOutput=False)
    d_kbdz = nc.declare_dram_parameter("kbdz", [128, 6 * KBD_W], F32R,
                                       isOutput=False)
    d_vbdz = nc.declare_dram_parameter("vbdz", [98, 6 * VBD_W], F32R,
                                       isOutput=False)
    d_out = nc.declare_dram_parameter("out", [TOK, C], F32, isOutput=True)

    # token tiles within a group
    tts = []
    o = 0
    while o < T:
        tts.append((o, min(128, T - o)))
        o += 128

    with tile.TileContext(nc) as tc, \
         nc.allow_low_precision(reason="float32r storage for full-rate matmul"):
        with tc.tile_pool(name="wres", bufs=1) as wres, \
             tc.tile_pool(name="xtm", bufs=4) as p_xtm, \
             tc.tile_pool(name="xT", bufs=1) as p_xT, \
             tc.tile_pool(name="qk", bufs=1) as p_qk, \
             tc.tile_pool(name="vscr", bufs=2) as p_vscr, \
             tc.tile_pool(name="eT", bufs=3) as p_eT, \
             tc.tile_pool(name="rr", bufs=1) as p_rr, \
             tc.tile_pool(name="bc", bufs=2) as p_bc, \
             tc.tile_pool(name="unT", bufs=1) as p_unT, \
             tc.tile_pool(name="osb", bufs=2) as p_osb, \
             tc.tile_pool(name="psA", bufs=2, space="PSUM") as psA, \
             tc.tile_pool(name="psB", bufs=2, space="PSUM") as psB, \
             tc.tile_pool(name="psS", bufs=2, space="PSUM") as psS, \
             tc.tile_pool(name="psO", bufs=2, space="PSUM") as psO:

            # ---- resident weights / constants ----
            w_qkv = []
            for ci in range(6):
                t = wres.tile([128, 3 * C], F32R, tag=f"wqkv{ci}")
                nc.sync.dma_start(t[:], d_wqkv[128 * ci:128 * (ci + 1), :])
                w_qkv.append(t)
            w_proj = []
            for ci in range(6):
                t = wres.tile([128, C], F32R, tag=f"wproj{ci}")
                nc.sync.dma_start(t[:], d_wproj[128 * ci:128 * (ci + 1), :])
                w_proj.append(t)
            ones1 = wres.tile([1, 128], F32R, tag="ones1")
            nc.sync.dma_start(ones1[:], d_ones1[:])
            sel2 = wres.tile([H, 6 * 128], F32R, tag="sel2")
            nc.sync.dma_start(sel2[:], d_sel2[:])
            onesbd = wres.tile([98, 2], F32R, tag="onesbd")
            nc.sync.dma_start(onesbd[:], d_onesbd[:])
            ident = wres.tile([128, 128], F32, tag="ident")
            nc.sync.dma_start(ident[:], d_ident[:])
            kbd = wres.tile([128, 6 * KBD_W], F32R, tag="kbd")
            nc.sync.dma_start(kbd[:], d_kbdz[:])
            vbd = wres.tile([98, 6 * VBD_W], F32R, tag="vbd")
            nc.sync.dma_start(vbd[:], d_vbdz[:])
            bias_sb = wres.tile([1, C], F32R, tag="bias_sb")
            nc.sync.dma_start(bias_sb[:], d_bias[:])
            bias_bc = wres.tile([128, C], F32, tag="bias_bc")
            for half in range(2):
                pb = psB.tile([128, 384], F32, tag="psB")
                nc.tensor.matmul(pb[:], ones1[:],
                                 bias_sb[:, 384 * half:384 * (half + 1)],
                                 start=True, stop=True)
                nc.scalar.copy(bias_bc[:, 384 * half:384 * (half + 1)], pb[:])

            for g in range(N_GROUPS):
                r0 = g * T  # first token row of group

                # ---- A: load x token-major, transpose to xT ----
                x_tm = []
                for (to, tk) in tts:
                    t = p_xtm.tile([128, C], F32, tag="xtm")
                    nc.sync.dma_start(t[:tk, :], d_x[r0 + to:r0 + to + tk, :])
                    x_tm.append(t)
                xT = [p_xT.tile([128, T], F32R, tag=f"xT{ci}", name=f"xT{ci}")
                      for ci in range(6)]
                for tti, (to, tk) in enumerate(tts):
                    for ci in range(6):
                        pt = psB.tile([128, 384], F32, tag="psB")
                        nc.tensor.transpose(
                            pt[:, :tk],
                            x_tm[tti][:tk, 128 * ci:128 * (ci + 1)],
                            ident[:tk, :tk])
                        nc.vector.tensor_copy(xT[ci][:, to:to + tk], pt[:, :tk])

                # ---- B: q,k GEMM; q natural, k scattered block-diag ----
                qk = []
                for j in range(12):
                    pq = psA.tile([128, TP], F32, tag="psA")
                    for ci in range(6):
                        nc.tensor.matmul(
                            pq[:, :T],
                            w_qkv[ci][:, 128 * j:128 * (j + 1)],
                            xT[ci][:, :T],
                            start=(ci == 0), stop=(ci == 5))
                    if j < 6:
                        t = p_qk.tile([128, TP], F32R, tag=f"q{j}", name=f"q{j}")
                        nc.vector.tensor_copy(t[:, :T], pq[:, :T])
                        nc.vector.tensor_copy(t[:, T:T + 2], pq[:, :2])
                        qk.append(t)
                    else:
                        jj = j - 6
                        kv = kbd[:, jj * KBD_W:(jj + 1) * KBD_W].rearrange(
                            "p (b c) -> p b c", c=98)
                        nc.vector.tensor_copy(
                            kv[0:64, :, 0:49],
                            pq[0:64, :T].rearrange("p (b c) -> p b c", c=49))
                        nc.vector.tensor_copy(
                            kv[64:128, :, 49:98],
                            pq[64:128, :T].rearrange("p (b c) -> p b c", c=49))

                # ---- C: v GEMM token-major + block-diag scatter ----
                v4 = vbd.rearrange("p (j b c) -> p j b c", b=G, c=128)
                for tti, (to, tk) in enumerate(tts):
                    scr = p_vscr.tile([128, C], F32R, tag="vscr")
                    for half in range(2):
                        pv = psB.tile([128, 384], F32, tag="psB")
                        for ci in range(6):
                            nc.tensor.matmul(
                                pv[:tk, :],
                                xT[ci][:, to:to + tk],
                                w_qkv[ci][:, 1536 + 384 * half:
                                           1536 + 384 * (half + 1)],
                                start=(ci == 0), stop=(ci == 5))
                        nc.vector.tensor_copy(
                            scr[:tk, 384 * half:384 * (half + 1)], pv[:tk, :])
                    # scatter batch segments of this token tile
                    for b in range(G):
                        lo = max(b * SEQ, to)
                        hi = min((b + 1) * SEQ, to + tk)
                        if lo >= hi:
                            continue
                        sl, sh = lo - b * SEQ, hi - b * SEQ  # rows in block
                        src = scr[lo - to:hi - to, :]
                        # even heads -> rows sl:sh, cols 0:64 of block
                        nc.sync.dma_start(
                            v4[sl:sh, :, b, 0:64],
                            src.rearrange("p (j two c) -> p j two c",
                                          two=2, c=64)[:, :, 0, :])
                        # odd heads -> rows 49+sl:49+sh, cols 64:128
                        nc.sync.dma_start(
                            v4[49 + sl:49 + sh, :, b, 64:128],
                            src.rearrange("p (j two c) -> p j two c",
                                          two=2, c=64)[:, :, 1, :])

                # ---- D: attention, one cell per head pair ----
                unT = [p_unT.tile([128, T], F32R, tag=f"unT{ci}", name=f"unT{ci}")
                       for ci in range(6)]
                rgrp = p_rr.tile([H, T], F32, tag="rgrp")
                for j in range(6):
                    ps = psS.tile([98, 50 * G], F32, tag="psS")
                    for b in range(G):
                        nc.tensor.matmul(
                            ps[:, 50 * b:50 * b + 50],
                            kbd[:, j * KBD_W + 98 * b:j * KBD_W + 98 * b + 98],
                            qk[j][:, 49 * b:49 * b + 50],
                            start=True, stop=True)
                    eT = p_eT.tile([98, 50 * G], F32R, tag="eT")
                    nc.scalar.activation(eT[:], ps[:], EXP, scale=0.125)
                    pr = psB.tile([2, 50 * G], F32, tag="psB")
                    nc.tensor.matmul(pr[:], onesbd[:], eT[:],
                                     start=True, stop=True)
                    po = psO.tile([128, 50 * G], F32, tag="psO")
                    for b in range(G):
                        nc.tensor.matmul(
                            po[:, 50 * b:50 * b + 50],
                            vbd[:, j * VBD_W + 128 * b:j * VBD_W + 128 * (b + 1)],
                            eT[:, 50 * b:50 * b + 50],
                            start=True, stop=True)
                    nc.vector.tensor_copy(
                        unT[j][:, :].rearrange("p (b c) -> p b c", c=49),
                        po[:, :].rearrange("p (b c) -> p b c", c=50)[:, :, 0:49])
                    r2 = p_bc.tile([2, T], F32, tag="r2")
                    nc.scalar.copy(
                        r2.rearrange("p (b c) -> p b c", c=49),
                        pr.rearrange("p (b c) -> p b c", c=50)[:, :, 0:49])
                    nc.sync.dma_start(rgrp[2 * j:2 * j + 2, :], r2[:])
                # one batched reciprocal for all 12 heads of the group
                rr = p_rr.tile([H, T], F32R, tag="rr")
                nc.vector.reciprocal(rr[:], rgrp[:])
                for j in range(6):
                    pbc = psA.tile([128, TP], F32, tag="psA")
                    nc.tensor.matmul(pbc[:, :T], sel2[:, 128 * j:128 * (j + 1)],
                                     rr[:], start=True, stop=True)
                    bc = p_bc.tile([128, T], F32, tag="bc")
                    nc.scalar.copy(bc[:], pbc[:, :T])
                    nc.vector.tensor_mul(out=unT[j][:, :], in0=unT[j][:, :],
                                         in1=bc[:])

                # ---- E: proj GEMM + bias -> out ----
                for tti, (to, tk) in enumerate(tts):
                    osb = p_osb.tile([128, C], F32, tag="osb")
                    for half in range(2):
                        pp = psB.tile([128, 384], F32, tag="psB")
                        for ci in range(6):
                            nc.tensor.matmul(
                                pp[:tk, :],
                                unT[ci][:, to:to + tk],
                                w_proj[ci][:, 384 * half:384 * (half + 1)],
                                start=(ci == 0), stop=(ci == 5))
                        nc.vector.tensor_add(
                            out=osb[:tk, 384 * half:384 * (half + 1)],
                            in0=pp[:tk, :],
                            in1=bias_bc[:tk, 384 * half:384 * (half + 1)])
                    nc.sync.dma_start(d_out[r0 + to:r0 + to + tk, :],
                                      osb[:tk, :])

    nc.compile()
    return nc


def kernel(x, W_qkv, W_proj, b_proj):
    from concourse.bass_utils import run_bass_kernel_spmd

    if "nc" not in _CACHE:
        _CACHE["nc"] = _build()
    nc = _CACHE["nc"]

    x = np.ascontiguousarray(np.asarray(x, dtype=np.float32))
    B, N, Cc = x.shape
    consts = _consts()
    W_qkv = np.ascontiguousarray(np.asarray(W_qkv, dtype=np.float32))
    W_proj = np.ascontiguousarray(np.asarray(W_proj, dtype=np.float32))
    bias = np.ascontiguousarray(
        np.asarray(b_proj, dtype=np.float32).reshape(1, Cc))

    in_maps = []
    for i in range(NUM_CORES):
        m = {"x": np.ascontiguousarray(
                x[i * B_CORE:(i + 1) * B_CORE].reshape(TOK, Cc)),
             "wqkv": W_qkv, "wproj": W_proj, "bias": bias}
        m.update(consts)
        in_maps.append(m)
    res = run_bass_kernel_spmd(nc, in_maps, list(range(NUM_CORES)))
    out = np.empty((B, N, Cc), dtype=np.float32)
    for i in range(NUM_CORES):
        out[i * B_CORE:(i + 1) * B_CORE] = res.results[i]["out"].reshape(
            B_CORE, N, Cc)
    return out



# revision 5
# speedup vs baseline: 1.7103x; 1.7103x over previous
"""Trainium2 Bass kernel: fused attention block (QKV proj -> MHA -> out proj).

Reference (per batch item b, NUM_HEADS=12, Dh=64):
    qkv = x @ W_qkv; q,k,v per head
    attn = softmax(q @ k^T / 8) @ v
    out  = concat_heads(attn) @ W_proj + b_proj

Sharding: data-parallel over batch across 8 NeuronCores (128 batch items
per core), weights replicated. One SPMD Bass program, per-core inputs.

v2 design (bf16 matmuls, feature-major I/O, half-array head pairing):
  - Host pre-transposes x to feature-major [C, TOK] bf16 and post-
    transposes the feature-major [C, TOK] bf16 output back; device never
    transposes anything (PE transposes eliminated entirely).
  - Groups of G=8 batches (T=392 tokens). Per group:
    B: q,k co-tiles [128, T] feature-major = Wqkv_slice.T @ xT (6-step
       ci accumulation, moving dim T=392).
    C: v token-major in 4 tiles of 98 tokens (2 batches each), scattered
       by SBUF->SBUF DMA into per-(head-pair, batch) blocks vbd2
       [128 rows, 64 hf]: rows 0:49 = even head kpos, 64:113 = odd head.
    D: per head-pair j: 8 batches x 2 half-array matmuls (rows/cols
       0:64 even head, 64:128 odd head via tile_position=(64,64)) for
       scores and for attn@V; exp on ACT; row sums via ones matmul;
       softmax normalization via reciprocal_approx_fast + broadcast
       matmul (selpair) + DVE multiply.
    E: out co-tiles [128, T] = Wproj_slice.T @ unT + bias (per-partition
       ACT bias add), DMA'd to feature-major output.
"""
import sys

sys.path.insert(0, "/opt/trn_rl_repo")

import numpy as np
import ml_dtypes

NUM_CORES = 8
B_CORE = 128          # batch items per core
SEQ = 49              # tokens per batch item
C = 768               # channels
H = 12                # heads
G = 8                 # batch items per group
T = SEQ * G           # 392 tokens per group
TOK = B_CORE * SEQ    # 6272 tokens per core
N_GROUPS = B_CORE // G

BF = ml_dtypes.bfloat16

_CACHE = {}


def _consts():
    onesbd = np.zeros((128, 2), dtype=BF)
    onesbd[0:49, 0] = 1.0
    onesbd[64:113, 1] = 1.0
    selpair = np.zeros((2, 128), dtype=BF)
    selpair[0, 0:64] = 1.0
    selpair[1, 64:128] = 1.0
    return {"onesbd": onesbd, "selpair": selpair}


def _build():
    import concourse.bacc as bacc
    import concourse.mybir as mybir
    import concourse.tile as tile

    F32 = mybir.dt.float32
    F32R = mybir.dt.float32r
    BF16 = mybir.dt.bfloat16
    EXP = mybir.ActivationFunctionType.Exp

    nc = bacc.Bacc("TRN2", target_bir_lowering=False)

    d_x = nc.declare_dram_parameter("x", [C, TOK], BF16, isOutput=False)
    d_wqkv = nc.declare_dram_parameter("wqkv", [C, 3 * C], BF16, isOutput=False)
    d_wproj = nc.declare_dram_parameter("wproj", [C, C], BF16, isOutput=False)
    d_bias = nc.declare_dram_parameter("bias", [128, 6], F32, isOutput=False)
    d_onesbd = nc.declare_dram_parameter("onesbd", [128, 2], BF16,
                                         isOutput=False)
    d_selpair = nc.declare_dram_parameter("selpair", [2, 128], BF16,
                                          isOutput=False)
    d_out = nc.declare_dram_parameter("out", [C, TOK], BF16, isOutput=True)

    with tile.TileContext(nc) as tc, \
         nc.allow_low_precision(reason="bf16 matmuls within 2e-2 tolerance"):
        with tc.tile_pool(name="wres", bufs=1) as wres, \
             tc.tile_pool(name="xT", bufs=2) as p_xT, \
             tc.tile_pool(name="qk", bufs=2) as p_qk, \
             tc.tile_pool(name="scr", bufs=3) as p_scr, \
             tc.tile_pool(name="vbd2", bufs=2) as p_vbd2, \
             tc.tile_pool(name="rr", bufs=3) as p_rr, \
             tc.tile_pool(name="unT", bufs=2) as p_unT, \
             tc.tile_pool(name="osb", bufs=2) as p_osb, \
             tc.tile_pool(name="psA", bufs=2, space="PSUM") as psA, \
             tc.tile_pool(name="psS", bufs=2, space="PSUM") as psS, \
             tc.tile_pool(name="psO", bufs=2, space="PSUM") as psO, \
             tc.tile_pool(name="psV", bufs=2, space="PSUM") as psV:

            # ---- resident weights / constants ----
            w_qkv = []
            for ci in range(6):
                t = wres.tile([128, 3 * C], BF16, tag=f"wqkv{ci}", name=f"wqkv{ci}")
                nc.sync.dma_start(t[:], d_wqkv[128 * ci:128 * (ci + 1), :])
                w_qkv.append(t)
            w_proj = []
            for ci in range(6):
                t = wres.tile([128, C], BF16, tag=f"wproj{ci}", name=f"wproj{ci}")
                nc.sync.dma_start(t[:], d_wproj[128 * ci:128 * (ci + 1), :])
                w_proj.append(t)
            onesbd = wres.tile([128, 2], BF16, tag="onesbd")
            nc.sync.dma_start(onesbd[:], d_onesbd[:])
            selpair = wres.tile([2, 128], BF16, tag="selpair")
            nc.sync.dma_start(selpair[:], d_selpair[:])
            bias_sb = wres.tile([128, 6], F32, tag="bias_sb")
            nc.sync.dma_start(bias_sb[:], d_bias[:])
            # exp tiles: dead bands (rows 49:64, 113:128) must be zero
            eTs = []
            for nm in ("eTa", "eTb"):
                t = wres.tile([128, T], BF16, tag=nm, name=nm)
                nc.vector.memset(t[:], 0.0)
                eTs.append(t)

            def load_xT(g):
                r0 = g * T
                xT = [p_xT.tile([128, T], BF16, tag=f"xT{ci}", name=f"xT{ci}")
                      for ci in range(6)]
                for ci in range(6):
                    nc.sync.dma_start(
                        xT[ci][:], d_x[128 * ci:128 * (ci + 1), r0:r0 + T])
                return xT

            xT_next = load_xT(0)
            for g in range(N_GROUPS):
                r0 = g * T
                xT = xT_next
                if g + 1 < N_GROUPS:
                    xT_next = load_xT(g + 1)

                # ---- B: q,k co-tiles feature-major ----
                q, k = [], []
                for j in range(12):
                    pq = psA.tile([128, T], F32, tag="psA", name="psA")
                    for ci in range(6):
                        nc.tensor.matmul(
                            pq[:], w_qkv[ci][:, 128 * j:128 * (j + 1)],
                            xT[ci][:], start=(ci == 0), stop=(ci == 5))
                    if j < 6:
                        t = p_qk.tile([128, T], BF16, tag=f"q{j}", name=f"q{j}")
                        nc.scalar.copy(t[:], pq[:])
                        q.append(t)
                    else:
                        t = p_qk.tile([128, T], BF16, tag=f"k{j - 6}", name=f"k{j - 6}")
                        nc.vector.tensor_copy(t[:], pq[:])
                        k.append(t)

                # ---- C: v token-major (98-token tiles) + scatter ----
                vbd2 = p_vbd2.tile([128, 6 * G * 64], BF16, tag="vbd2", name="vbd2")
                v4 = vbd2.rearrange("p (j b c) -> p j b c", b=G, c=64)
                for p4 in range(4):
                    to = 98 * p4
                    scr = p_scr.tile([98, C], BF16, tag="scr", name="scr")
                    for half in range(2):
                        pv = psV.tile([98, 384], F32, tag="psV", name="psV")
                        for ci in range(6):
                            nc.tensor.matmul(
                                pv[:], xT[ci][:, to:to + 98],
                                w_qkv[ci][:, 1536 + 384 * half:
                                           1536 + 384 * (half + 1)],
                                start=(ci == 0), stop=(ci == 5))
                        nc.vector.tensor_copy(
                            scr[:, 384 * half:384 * (half + 1)], pv[:])
                    sv = scr.rearrange("p (j two c) -> p j two c", two=2, c=64)
                    for loc, b in ((0, 2 * p4), (49, 2 * p4 + 1)):
                        nc.sync.dma_start(v4[0:49, :, b, :],
                                          sv[loc:loc + 49, :, 0, :])
                        nc.sync.dma_start(v4[64:113, :, b, :],
                                          sv[loc:loc + 49, :, 1, :])

                # ---- D: attention per head pair, 2-stage pipeline ----
                unT = [p_unT.tile([128, T], BF16, tag=f"unT{ci}", name=f"unT{ci}")
                       for ci in range(6)]
                stash = {}

                def d_head(j):
                    eT = eTs[j % 2]
                    ps = psS.tile([128, T], F32, tag="psS", name="psS")
                    for b in range(G):
                        bs = slice(49 * b, 49 * b + 49)
                        nc.tensor.matmul(ps[0:49, bs], k[j][0:64, bs],
                                         q[j][0:64, bs],
                                         start=True, stop=True)
                        nc.tensor.matmul(ps[64:113, bs], k[j][64:128, bs],
                                         q[j][64:128, bs],
                                         start=True, stop=True,
                                         tile_position=(64, 64))
                    nc.scalar.activation(eT[0:49, :], ps[0:49, :], EXP,
                                         scale=0.125)
                    nc.scalar.activation(eT[64:113, :], ps[64:113, :], EXP,
                                         scale=0.125)
                    stash[j] = (eT, ps)

                def d_tail(j):
                    eT, ps = stash.pop(j)
                    # row sums into rows 0:2 of the scores psum (scores
                    # already consumed by exp); reciprocal runs on DVE
                    # while the attn@V matmuls stream
                    nc.tensor.matmul(ps[0:2, :], onesbd[:], eT[:],
                                     start=True, stop=True)
                    rr = p_rr.tile([2, T], F32, tag="rr", name="rr")
                    nc.vector.reciprocal_approx_fast(rr[:], ps[0:2, :])
                    rrb = p_rr.tile([2, T], BF16, tag="rrb", name="rrb")
                    nc.gpsimd.tensor_copy(rrb[:], rr[:])
                    po = psO.tile([128, T], F32, tag="psO", name="psO")
                    for b in range(G):
                        bs = slice(49 * b, 49 * b + 49)
                        nc.tensor.matmul(po[0:64, bs], v4[0:49, j, b, :],
                                         eT[0:49, bs], start=True, stop=True)
                        nc.tensor.matmul(po[64:128, bs], v4[64:113, j, b, :],
                                         eT[64:113, bs],
                                         start=True, stop=True,
                                         tile_position=(64, 64))
                    # broadcast 1/rowsum across the 2 head halves
                    nc.tensor.matmul(ps[:], selpair[:], rrb[:],
                                     start=True, stop=True)
                    nc.vector.tensor_copy(unT[j][:], po[:])
                    nc.vector.tensor_mul(out=unT[j][:], in0=unT[j][:],
                                         in1=ps[:])

                for j in range(6):
                    d_head(j)
                    if j >= 1:
                        d_tail(j - 1)
                d_tail(5)

                # ---- E: proj co-tiles feature-major + bias ----
                for j2 in range(6):
                    pp = psA.tile([128, T], F32, tag="psA", name="psA")
                    for ci in range(6):
                        nc.tensor.matmul(
                            pp[:], w_proj[ci][:, 128 * j2:128 * (j2 + 1)],
                            unT[ci][:], start=(ci == 0), stop=(ci == 5))
                    osb = p_osb.tile([128, T], BF16, tag="osb", name="osb")
                    nc.scalar.add(osb[:], pp[:], bias_sb[:, j2:j2 + 1])
                    nc.sync.dma_start(
                        d_out[128 * j2:128 * (j2 + 1), r0:r0 + T], osb[:])

    nc.compile()
    return nc


def _prep_inputs(x, W_qkv, W_proj, b_proj):
    x = np.asarray(x, dtype=np.float32)
    B, N, Cc = x.shape
    consts = _consts()
    wqkv = np.ascontiguousarray(np.asarray(W_qkv, dtype=np.float32)).astype(BF)
    wproj = np.ascontiguousarray(np.asarray(W_proj, dtype=np.float32)).astype(BF)
    bias = np.ascontiguousarray(
        np.asarray(b_proj, dtype=np.float32).reshape(6, 128).T)
    x_bf = x.astype(BF)
    in_maps = []
    for i in range(NUM_CORES):
        xt = np.ascontiguousarray(
            x_bf[i * B_CORE:(i + 1) * B_CORE].reshape(TOK, Cc).T)
        m = {"x": xt, "wqkv": wqkv, "wproj": wproj, "bias": bias}
        m.update(consts)
        in_maps.append(m)
    return in_maps


def _unshard(results):
    out = np.empty((NUM_CORES * B_CORE, SEQ, C), dtype=np.float32)
    for i in range(NUM_CORES):
        o = np.asarray(results[i]["out"]).astype(np.float32)  # [C, TOK]
        out[i * B_CORE:(i + 1) * B_CORE] = o.T.reshape(B_CORE, SEQ, C)
    return out


def kernel(x, W_qkv, W_proj, b_proj):
    from concourse.bass_utils import run_bass_kernel_spmd

    if "nc" not in _CACHE:
        _CACHE["nc"] = _build()
    nc = _CACHE["nc"]

    in_maps = _prep_inputs(x, W_qkv, W_proj, b_proj)
    res = run_bass_kernel_spmd(nc, in_maps, list(range(NUM_CORES)))
    return _unshard(res.results)


# revision 9
# speedup vs baseline: 2.0401x; 1.1928x over previous
"""Trainium2 Bass kernel: fused attention block (QKV proj -> MHA -> out proj).

Reference (per batch item b, NUM_HEADS=12, Dh=64):
    qkv = x @ W_qkv; q,k,v per head
    attn = softmax(q @ k^T / 8) @ v
    out  = concat_heads(attn) @ W_proj + b_proj

Sharding: data-parallel over batch across 8 NeuronCores (128 batch items
per core), weights replicated. One SPMD Bass program, per-core inputs.

v2 design (bf16 matmuls, feature-major I/O, half-array head pairing):
  - Host pre-transposes x to feature-major [C, TOK] bf16 and post-
    transposes the feature-major [C, TOK] bf16 output back; device never
    transposes anything (PE transposes eliminated entirely).
  - Groups of G=8 batches (T=392 tokens). Per group:
    B: q,k co-tiles [128, T] feature-major = Wqkv_slice.T @ xT (6-step
       ci accumulation, moving dim T=392).
    C: v token-major in 4 tiles of 98 tokens (2 batches each), scattered
       by SBUF->SBUF DMA into per-(head-pair, batch) blocks vbd2
       [128 rows, 64 hf]: rows 0:49 = even head kpos, 64:113 = odd head.
    D: per head-pair j: 8 batches x 2 half-array matmuls (rows/cols
       0:64 even head, 64:128 odd head via tile_position=(64,64)) for
       scores and for attn@V; exp on ACT; row sums via ones matmul;
       softmax normalization via reciprocal_approx_fast + broadcast
       matmul (selpair) + DVE multiply.
    E: out co-tiles [128, T] = Wproj_slice.T @ unT + bias (per-partition
       ACT bias add), DMA'd to feature-major output.
"""
import sys

sys.path.insert(0, "/opt/trn_rl_repo")

import numpy as np
import ml_dtypes

NUM_CORES = 8
B_CORE = 128          # batch items per core
SEQ = 49              # tokens per batch item
C = 768               # channels
H = 12                # heads
G = 8                 # batch items per group
T = SEQ * G           # 392 tokens per group
TOK = B_CORE * SEQ    # 6272 tokens per core
N_GROUPS = B_CORE // G

BF = ml_dtypes.bfloat16

_CACHE = {}


def _consts():
    onesbd = np.zeros((128, 2), dtype=BF)
    onesbd[0:49, 0] = 1.0
    onesbd[64:113, 1] = 1.0
    selpair = np.zeros((2, 128), dtype=BF)
    selpair[0, 0:64] = 1.0
    selpair[1, 64:128] = 1.0
    return {"onesbd": onesbd, "selpair": selpair}


def _build():
    import concourse.bacc as bacc
    import concourse.mybir as mybir
    import concourse.tile as tile

    F32 = mybir.dt.float32
    F32R = mybir.dt.float32r
    BF16 = mybir.dt.bfloat16
    EXP = mybir.ActivationFunctionType.Exp

    nc = bacc.Bacc("TRN2", target_bir_lowering=False)

    d_x = nc.declare_dram_parameter("x", [C, TOK], BF16, isOutput=False)
    d_wqkv = nc.declare_dram_parameter("wqkv", [C, 3 * C], BF16, isOutput=False)
    d_wproj = nc.declare_dram_parameter("wproj", [C, C], BF16, isOutput=False)
    d_bias = nc.declare_dram_parameter("bias", [128, 6], F32, isOutput=False)
    d_onesbd = nc.declare_dram_parameter("onesbd", [128, 2], BF16,
                                         isOutput=False)
    d_selpair = nc.declare_dram_parameter("selpair", [2, 128], BF16,
                                          isOutput=False)
    d_out = nc.declare_dram_parameter("out", [C, TOK], BF16, isOutput=True)

    with tile.TileContext(nc) as tc, \
         nc.allow_low_precision(reason="bf16 matmuls within 2e-2 tolerance"):
        with tc.tile_pool(name="wres", bufs=1) as wres, \
             tc.tile_pool(name="xT", bufs=2) as p_xT, \
             tc.tile_pool(name="qk", bufs=2) as p_qk, \
             tc.tile_pool(name="scr", bufs=3) as p_scr, \
             tc.tile_pool(name="vbd2", bufs=2) as p_vbd2, \
             tc.tile_pool(name="rr", bufs=3) as p_rr, \
             tc.tile_pool(name="unT", bufs=2) as p_unT, \
             tc.tile_pool(name="osb", bufs=2) as p_osb, \
             tc.tile_pool(name="psA", bufs=2, space="PSUM") as psA, \
             tc.tile_pool(name="psS", bufs=2, space="PSUM") as psS, \
             tc.tile_pool(name="psO", bufs=2, space="PSUM") as psO, \
             tc.tile_pool(name="psV", bufs=2, space="PSUM") as psV:

            # ---- resident weights / constants ----
            w_qkv = []
            for ci in range(6):
                t = wres.tile([128, 3 * C], BF16, tag=f"wqkv{ci}", name=f"wqkv{ci}")
                nc.sync.dma_start(t[:], d_wqkv[128 * ci:128 * (ci + 1), :])
                w_qkv.append(t)
            def load_xT(g):
                r0 = g * T
                xT = [p_xT.tile([128, T], BF16, tag=f"xT{ci}", name=f"xT{ci}")
                      for ci in range(6)]
                for ci in range(6):
                    nc.sync.dma_start(
                        xT[ci][:], d_x[128 * ci:128 * (ci + 1), r0:r0 + T])
                return xT

            # group-0 x tiles right after W_qkv: the first B-stage matmuls
            # need only these, so the PE starts as early as possible
            xT_next = load_xT(0)
            w_proj = []
            for ci in range(6):
                t = wres.tile([128, C], BF16, tag=f"wproj{ci}", name=f"wproj{ci}")
                nc.sync.dma_start(t[:], d_wproj[128 * ci:128 * (ci + 1), :])
                w_proj.append(t)
            onesbd = wres.tile([128, 2], BF16, tag="onesbd")
            nc.sync.dma_start(onesbd[:], d_onesbd[:])
            selpair = wres.tile([2, 128], BF16, tag="selpair")
            nc.sync.dma_start(selpair[:], d_selpair[:])
            bias_sb = wres.tile([128, 6], F32, tag="bias_sb")
            nc.sync.dma_start(bias_sb[:], d_bias[:])
            # exp tiles: dead bands (rows 49:64, 113:128) must be zero
            eTs = []
            for nm in ("eTa", "eTb"):
                t = wres.tile([128, T], BF16, tag=nm, name=nm)
                nc.vector.memset(t[:], 0.0)
                eTs.append(t)

            e_prev = None
            for g in range(N_GROUPS):
                r0 = g * T
                xT = xT_next
                if g + 1 < N_GROUPS:
                    xT_next = load_xT(g + 1)

                # ---- B: q,k co-tiles feature-major ----
                q, k = [], []
                for j in range(12):
                    pq = psA.tile([128, T], F32, tag="psA", name="psA")
                    for ci in range(6):
                        nc.tensor.matmul(
                            pq[:], w_qkv[ci][:, 128 * j:128 * (j + 1)],
                            xT[ci][:], start=(ci == 0), stop=(ci == 5))
                    if j < 6:
                        t = p_qk.tile([128, T], BF16, tag=f"q{j}", name=f"q{j}")
                        nc.scalar.copy(t[:], pq[:])
                        q.append(t)
                    else:
                        t = p_qk.tile([128, T], BF16, tag=f"k{j - 6}", name=f"k{j - 6}")
                        nc.vector.tensor_copy(t[:], pq[:])
                        k.append(t)

                # ---- C: v token-major (98-token tiles) + scatter ----
                vbd2 = p_vbd2.tile([128, 6 * G * 64], BF16, tag="vbd2", name="vbd2")
                v4 = vbd2.rearrange("p (j b c) -> p j b c", b=G, c=64)
                for p4 in range(4):
                    to = 98 * p4
                    scr = p_scr.tile([98, C], BF16, tag="scr", name="scr")
                    for half in range(2):
                        pv = psV.tile([98, 384], F32, tag="psV", name="psV")
                        for ci in range(6):
                            nc.tensor.matmul(
                                pv[:], xT[ci][:, to:to + 98],
                                w_qkv[ci][:, 1536 + 384 * half:
                                           1536 + 384 * (half + 1)],
                                start=(ci == 0), stop=(ci == 5))
                        nc.vector.tensor_copy(
                            scr[:, 384 * half:384 * (half + 1)], pv[:])
                    sv = scr.rearrange("p (j two c) -> p j two c", two=2, c=64)
                    for loc, b in ((0, 2 * p4), (49, 2 * p4 + 1)):
                        nc.sync.dma_start(v4[0:49, :, b, :],
                                          sv[loc:loc + 49, :, 0, :])
                        nc.sync.dma_start(v4[64:113, :, b, :],
                                          sv[loc:loc + 49, :, 1, :])

                # ---- D: attention per head pair, 2-stage pipeline ----
                unT = [p_unT.tile([128, T], BF16, tag=f"unT{ci}", name=f"unT{ci}")
                       for ci in range(6)]
                stash = {}

                def d_head(j):
                    eT = eTs[j % 2]
                    ps = psS.tile([128, T], F32, tag="psS", name="psS")
                    for b in range(G):
                        bs = slice(49 * b, 49 * b + 49)
                        nc.tensor.matmul(ps[0:49, bs], k[j][0:64, bs],
                                         q[j][0:64, bs],
                                         start=True, stop=True)
                        nc.tensor.matmul(ps[64:113, bs], k[j][64:128, bs],
                                         q[j][64:128, bs],
                                         start=True, stop=True,
                                         tile_position=(64, 64))
                    nc.scalar.activation(eT[0:49, :], ps[0:49, :], EXP,
                                         scale=0.125)
                    nc.scalar.activation(eT[64:113, :], ps[64:113, :], EXP,
                                         scale=0.125)
                    stash[j] = (eT, ps)

                def d_tail(j):
                    eT, ps = stash.pop(j)
                    # row sums into rows 0:2 of the scores psum (scores
                    # already consumed by exp); reciprocal runs on DVE
                    # while the attn@V matmuls stream
                    nc.tensor.matmul(ps[0:2, :], onesbd[:], eT[:],
                                     start=True, stop=True)
                    rr = p_rr.tile([2, T], F32, tag="rr", name="rr")
                    nc.vector.reciprocal_approx_fast(rr[:], ps[0:2, :])
                    rrb = p_rr.tile([2, T], BF16, tag="rrb", name="rrb")
                    nc.vector.tensor_copy(rrb[:], rr[:])
                    po = psO.tile([128, T], F32, tag="psO", name="psO")
                    for b in range(G):
                        bs = slice(49 * b, 49 * b + 49)
                        nc.tensor.matmul(po[0:64, bs], v4[0:49, j, b, :],
                                         eT[0:49, bs], start=True, stop=True)
                        nc.tensor.matmul(po[64:128, bs], v4[64:113, j, b, :],
                                         eT[64:113, bs],
                                         start=True, stop=True,
                                         tile_position=(64, 64))
                    # broadcast 1/rowsum across the 2 head halves
                    nc.tensor.matmul(ps[:], selpair[:], rrb[:],
                                     start=True, stop=True)
                    nc.vector.tensor_copy(unT[j][:], po[:])
                    nc.vector.tensor_mul(out=unT[j][:], in0=unT[j][:],
                                         in1=ps[:])

                # interleave previous group's E-stage (dense 392-wide
                # matmuls) into this group's D-stage so the PE never idles
                # long enough for HAM to re-throttle
                for j in range(6):
                    d_head(j)
                    if j >= 1:
                        d_tail(j - 1)
                    if e_prev is not None:
                        e_prev(j)
                d_tail(5)

                def make_e(r0_, unT_):
                    def e_stage(j2):
                        pp = psO.tile([128, T], F32, tag="psO", name="psO")
                        for ci in range(6):
                            nc.tensor.matmul(
                                pp[:], w_proj[ci][:, 128 * j2:128 * (j2 + 1)],
                                unT_[ci][:], start=(ci == 0), stop=(ci == 5))
                        osb = p_osb.tile([128, T], BF16, tag="osb",
                                         name="osb")
                        nc.scalar.add(osb[:], pp[:], bias_sb[:, j2:j2 + 1])
                        nc.sync.dma_start(
                            d_out[128 * j2:128 * (j2 + 1), r0_:r0_ + T],
                            osb[:])
                    return e_stage

                e_prev = make_e(r0, unT)

            # E-stage of the final group
            for j2 in range(6):
                e_prev(j2)

    nc.compile()
    return nc


def _prep_inputs(x, W_qkv, W_proj, b_proj):
    x = np.asarray(x, dtype=np.float32)
    B, N, Cc = x.shape
    consts = _consts()
    wqkv = np.ascontiguousarray(np.asarray(W_qkv, dtype=np.float32)).astype(BF)
    wproj = np.ascontiguousarray(np.asarray(W_proj, dtype=np.float32)).astype(BF)
    bias = np.ascontiguousarray(
        np.asarray(b_proj, dtype=np.float32).reshape(6, 128).T)
    x_bf = x.astype(BF)
    in_maps = []
    for i in range(NUM_CORES):
        xt = np.ascontiguousarray(
            x_bf[i * B_CORE:(i + 1) * B_CORE].reshape(TOK, Cc).T)
        m = {"x": xt, "wqkv": wqkv, "wproj": wproj, "bias": bias}
        m.update(consts)
        in_maps.append(m)
    return in_maps


def _unshard(results):
    out = np.empty((NUM_CORES * B_CORE, SEQ, C), dtype=np.float32)
    for i in range(NUM_CORES):
        o = np.asarray(results[i]["out"]).astype(np.float32)  # [C, TOK]
        out[i * B_CORE:(i + 1) * B_CORE] = o.T.reshape(B_CORE, SEQ, C)
    return out


def kernel(x, W_qkv, W_proj, b_proj):
    from concourse.bass_utils import run_bass_kernel_spmd

    if "nc" not in _CACHE:
        _CACHE["nc"] = _build()
    nc = _CACHE["nc"]

    in_maps = _prep_inputs(x, W_qkv, W_proj, b_proj)
    res = run_bass_kernel_spmd(nc, in_maps, list(range(NUM_CORES)))
    return _unshard(res.results)


# revision 10
# speedup vs baseline: 2.1032x; 1.0309x over previous
"""Trainium2 Bass kernel: fused attention block (QKV proj -> MHA -> out proj).

Reference (per batch item b, NUM_HEADS=12, Dh=64):
    qkv = x @ W_qkv; q,k,v per head
    attn = softmax(q @ k^T / 8) @ v
    out  = concat_heads(attn) @ W_proj + b_proj

Sharding: data-parallel over batch across 8 NeuronCores (128 batch items
per core), weights replicated. One SPMD Bass program, per-core inputs.

v2 design (bf16 matmuls, feature-major I/O, half-array head pairing):
  - Host pre-transposes x to feature-major [C, TOK] bf16 and post-
    transposes the feature-major [C, TOK] bf16 output back; device never
    transposes anything (PE transposes eliminated entirely).
  - Groups of G=8 batches (T=392 tokens). Per group:
    B: q,k co-tiles [128, T] feature-major = Wqkv_slice.T @ xT (6-step
       ci accumulation, moving dim T=392).
    C: v token-major in 4 tiles of 98 tokens (2 batches each), scattered
       by SBUF->SBUF DMA into per-(head-pair, batch) blocks vbd2
       [128 rows, 64 hf]: rows 0:49 = even head kpos, 64:113 = odd head.
    D: per head-pair j: 8 batches x 2 half-array matmuls (rows/cols
       0:64 even head, 64:128 odd head via tile_position=(64,64)) for
       scores and for attn@V; exp on ACT; row sums via ones matmul;
       softmax normalization via reciprocal_approx_fast + broadcast
       matmul (selpair) + DVE multiply.
    E: out co-tiles [128, T] = Wproj_slice.T @ unT + bias (per-partition
       ACT bias add), DMA'd to feature-major output.
"""
import sys

sys.path.insert(0, "/opt/trn_rl_repo")

import numpy as np
import ml_dtypes

NUM_CORES = 8
B_CORE = 128          # batch items per core
SEQ = 49              # tokens per batch item
C = 768               # channels
H = 12                # heads
G = 8                 # batch items per group
T = SEQ * G           # 392 tokens per group
TOK = B_CORE * SEQ    # 6272 tokens per core
N_GROUPS = B_CORE // G

BF = ml_dtypes.bfloat16

_CACHE = {}


def _consts():
    onesbd = np.zeros((128, 2), dtype=BF)
    onesbd[0:49, 0] = 1.0
    onesbd[64:113, 1] = 1.0
    selpair = np.zeros((2, 128), dtype=BF)
    selpair[0, 0:64] = 1.0
    selpair[1, 64:128] = 1.0
    return {"onesbd": onesbd, "selpair": selpair}


def _build():
    import concourse.bacc as bacc
    import concourse.mybir as mybir
    import concourse.tile as tile

    F32 = mybir.dt.float32
    F32R = mybir.dt.float32r
    BF16 = mybir.dt.bfloat16
    EXP = mybir.ActivationFunctionType.Exp

    nc = bacc.Bacc("TRN2", target_bir_lowering=False)

    d_x = nc.declare_dram_parameter("x", [C, TOK], BF16, isOutput=False)
    d_wqkv = nc.declare_dram_parameter("wqkv", [C, 3 * C], BF16, isOutput=False)
    d_wproj = nc.declare_dram_parameter("wproj", [C, C], BF16, isOutput=False)
    d_bias = nc.declare_dram_parameter("bias", [128, 6], F32, isOutput=False)
    d_onesbd = nc.declare_dram_parameter("onesbd", [128, 2], BF16,
                                         isOutput=False)
    d_selpair = nc.declare_dram_parameter("selpair", [2, 128], BF16,
                                          isOutput=False)
    d_out = nc.declare_dram_parameter("out", [C, TOK], BF16, isOutput=True)

    with tile.TileContext(nc) as tc, \
         nc.allow_low_precision(reason="bf16 matmuls within 2e-2 tolerance"):
        with tc.tile_pool(name="wres", bufs=1) as wres, \
             tc.tile_pool(name="xT", bufs=2) as p_xT, \
             tc.tile_pool(name="qk", bufs=2) as p_qk, \
             tc.tile_pool(name="scr", bufs=3) as p_scr, \
             tc.tile_pool(name="vbd2", bufs=2) as p_vbd2, \
             tc.tile_pool(name="rr", bufs=3) as p_rr, \
             tc.tile_pool(name="unT", bufs=2) as p_unT, \
             tc.tile_pool(name="osb", bufs=2) as p_osb, \
             tc.tile_pool(name="psA", bufs=2, space="PSUM") as psA, \
             tc.tile_pool(name="psS", bufs=2, space="PSUM") as psS, \
             tc.tile_pool(name="psO", bufs=2, space="PSUM") as psO, \
             tc.tile_pool(name="psV", bufs=2, space="PSUM") as psV:

            # ---- resident weights / constants ----
            w_qkv = []
            for ci in range(6):
                t = wres.tile([128, 3 * C], BF16, tag=f"wqkv{ci}", name=f"wqkv{ci}")
                nc.sync.dma_start(t[:, 0:1536],
                                  d_wqkv[128 * ci:128 * (ci + 1), 0:1536])
                w_qkv.append(t)
            def load_xT(g):
                r0 = g * T
                xT = [p_xT.tile([128, T], BF16, tag=f"xT{ci}", name=f"xT{ci}")
                      for ci in range(6)]
                for ci in range(6):
                    nc.sync.dma_start(
                        xT[ci][:], d_x[128 * ci:128 * (ci + 1), r0:r0 + T])
                return xT

            # group-0 x tiles right after W_qkv's q,k columns: the first
            # B-stage matmuls need only these, so the PE starts early
            xT_next = load_xT(0)
            for ci in range(6):
                nc.sync.dma_start(w_qkv[ci][:, 1536:2304],
                                  d_wqkv[128 * ci:128 * (ci + 1), 1536:2304])
            w_proj = []
            for ci in range(6):
                t = wres.tile([128, C], BF16, tag=f"wproj{ci}", name=f"wproj{ci}")
                nc.sync.dma_start(t[:], d_wproj[128 * ci:128 * (ci + 1), :])
                w_proj.append(t)
            onesbd = wres.tile([128, 2], BF16, tag="onesbd")
            nc.sync.dma_start(onesbd[:], d_onesbd[:])
            selpair = wres.tile([2, 128], BF16, tag="selpair")
            nc.sync.dma_start(selpair[:], d_selpair[:])
            bias_sb = wres.tile([128, 6], F32, tag="bias_sb")
            nc.sync.dma_start(bias_sb[:], d_bias[:])
            # exp tiles: dead bands (rows 49:64, 113:128) must be zero
            eTs = []
            for nm in ("eTa", "eTb"):
                t = wres.tile([128, T], BF16, tag=nm, name=nm)
                nc.vector.memset(t[:], 0.0)
                eTs.append(t)

            e_prev = None
            for g in range(N_GROUPS):
                r0 = g * T
                xT = xT_next
                if g + 1 < N_GROUPS:
                    xT_next = load_xT(g + 1)

                # ---- B: q,k co-tiles feature-major ----
                q, k = [], []
                for j in range(12):
                    pq = psA.tile([128, T], F32, tag="psA", name="psA")
                    for ci in range(6):
                        nc.tensor.matmul(
                            pq[:], w_qkv[ci][:, 128 * j:128 * (j + 1)],
                            xT[ci][:], start=(ci == 0), stop=(ci == 5))
                    if j < 6:
                        t = p_qk.tile([128, T], BF16, tag=f"q{j}", name=f"q{j}")
                        nc.scalar.copy(t[:], pq[:])
                        q.append(t)
                    else:
                        t = p_qk.tile([128, T], BF16, tag=f"k{j - 6}", name=f"k{j - 6}")
                        nc.vector.tensor_copy(t[:], pq[:])
                        k.append(t)

                # ---- C: v token-major (98-token tiles) + scatter ----
                vbd2 = p_vbd2.tile([128, 6 * G * 64], BF16, tag="vbd2", name="vbd2")
                v4 = vbd2.rearrange("p (j b c) -> p j b c", b=G, c=64)
                for p4 in range(4):
                    to = 98 * p4
                    scr = p_scr.tile([98, C], BF16, tag="scr", name="scr")
                    for half in range(2):
                        pv = psV.tile([98, 384], F32, tag="psV", name="psV")
                        for ci in range(6):
                            nc.tensor.matmul(
                                pv[:], xT[ci][:, to:to + 98],
                                w_qkv[ci][:, 1536 + 384 * half:
                                           1536 + 384 * (half + 1)],
                                start=(ci == 0), stop=(ci == 5))
                        nc.vector.tensor_copy(
                            scr[:, 384 * half:384 * (half + 1)], pv[:])
                    sv = scr.rearrange("p (j two c) -> p j two c", two=2, c=64)
                    for loc, b in ((0, 2 * p4), (49, 2 * p4 + 1)):
                        nc.sync.dma_start(v4[0:49, :, b, :],
                                          sv[loc:loc + 49, :, 0, :])
                        nc.sync.dma_start(v4[64:113, :, b, :],
                                          sv[loc:loc + 49, :, 1, :])

                # ---- D: attention per head pair, 2-stage pipeline ----
                unT = [p_unT.tile([128, T], BF16, tag=f"unT{ci}", name=f"unT{ci}")
                       for ci in range(6)]
                stash = {}

                def d_head(j):
                    eT = eTs[j % 2]
                    ps = psS.tile([128, T], F32, tag="psS", name="psS")
                    for b in range(G):
                        bs = slice(49 * b, 49 * b + 49)
                        nc.tensor.matmul(ps[0:49, bs], k[j][0:64, bs],
                                         q[j][0:64, bs],
                                         start=True, stop=True)
                        nc.tensor.matmul(ps[64:113, bs], k[j][64:128, bs],
                                         q[j][64:128, bs],
                                         start=True, stop=True,
                                         tile_position=(64, 64))
                    if g == 0 and j < 2:
                        # first use of each psS slot: rows 49:64 are
                        # uninitialized psum, so exp only the live bands
                        nc.scalar.activation(eT[0:49, :], ps[0:49, :], EXP,
                                             scale=0.125)
                        nc.scalar.activation(eT[64:113, :], ps[64:113, :],
                                             EXP, scale=0.125)
                    else:
                        # single ACT op over rows 0:113; rows 49:64 hold
                        # stale-but-finite bc values from a previous j-step
                        # and exp of them is multiplied by onesbd zeros
                        nc.scalar.activation(eT[0:113, :], ps[0:113, :], EXP,
                                             scale=0.125)
                    stash[j] = (eT, ps)

                def d_tail(j):
                    eT, ps = stash.pop(j)
                    # row sums into rows 0:2 of the scores psum (scores
                    # already consumed by exp); reciprocal runs on DVE
                    # while the attn@V matmuls stream
                    nc.tensor.matmul(ps[0:2, :], onesbd[:], eT[:],
                                     start=True, stop=True)
                    rr = p_rr.tile([2, T], F32, tag="rr", name="rr")
                    nc.vector.reciprocal_approx_fast(rr[:], ps[0:2, :])
                    rrb = p_rr.tile([2, T], BF16, tag="rrb", name="rrb")
                    nc.vector.tensor_copy(rrb[:], rr[:])
                    po = psO.tile([128, T], F32, tag="psO", name="psO")
                    for b in range(G):
                        bs = slice(49 * b, 49 * b + 49)
                        nc.tensor.matmul(po[0:64, bs], v4[0:49, j, b, :],
                                         eT[0:49, bs], start=True, stop=True)
                        nc.tensor.matmul(po[64:128, bs], v4[64:113, j, b, :],
                                         eT[64:113, bs],
                                         start=True, stop=True,
                                         tile_position=(64, 64))
                    # broadcast 1/rowsum across the 2 head halves
                    nc.tensor.matmul(ps[:], selpair[:], rrb[:],
                                     start=True, stop=True)
                    nc.scalar.copy(unT[j][:], po[:])
                    nc.vector.tensor_mul(out=unT[j][:], in0=unT[j][:],
                                         in1=ps[:])

                # interleave previous group's E-stage (dense 392-wide
                # matmuls) into this group's D-stage so the PE never idles
                # long enough for HAM to re-throttle
                for j in range(6):
                    d_head(j)
                    if j >= 1:
                        d_tail(j - 1)
                    if e_prev is not None:
                        e_prev(j)
                d_tail(5)

                def make_e(r0_, unT_):
                    def e_stage(j2):
                        pp = psO.tile([128, T], F32, tag="psO", name="psO")
                        for ci in range(6):
                            nc.tensor.matmul(
                                pp[:], w_proj[ci][:, 128 * j2:128 * (j2 + 1)],
                                unT_[ci][:], start=(ci == 0), stop=(ci == 5))
                        osb = p_osb.tile([128, T], BF16, tag="osb",
                                         name="osb")
                        nc.scalar.add(osb[:], pp[:], bias_sb[:, j2:j2 + 1])
                        nc.sync.dma_start(
                            d_out[128 * j2:128 * (j2 + 1), r0_:r0_ + T],
                            osb[:])
                    return e_stage

                e_prev = make_e(r0, unT)

            # E-stage of the final group
            for j2 in range(6):
                e_prev(j2)

    nc.compile()
    return nc


def _prep_inputs(x, W_qkv, W_proj, b_proj):
    x = np.asarray(x, dtype=np.float32)
    B, N, Cc = x.shape
    consts = _consts()
    wqkv = np.ascontiguousarray(np.asarray(W_qkv, dtype=np.float32)).astype(BF)
    wproj = np.ascontiguousarray(np.asarray(W_proj, dtype=np.float32)).astype(BF)
    bias = np.ascontiguousarray(
        np.asarray(b_proj, dtype=np.float32).reshape(6, 128).T)
    x_bf = x.astype(BF)
    in_maps = []
    for i in range(NUM_CORES):
        xt = np.ascontiguousarray(
            x_bf[i * B_CORE:(i + 1) * B_CORE].reshape(TOK, Cc).T)
        m = {"x": xt, "wqkv": wqkv, "wproj": wproj, "bias": bias}
        m.update(consts)
        in_maps.append(m)
    return in_maps


def _unshard(results):
    out = np.empty((NUM_CORES * B_CORE, SEQ, C), dtype=np.float32)
    for i in range(NUM_CORES):
        o = np.asarray(results[i]["out"]).astype(np.float32)  # [C, TOK]
        out[i * B_CORE:(i + 1) * B_CORE] = o.T.reshape(B_CORE, SEQ, C)
    return out


def kernel(x, W_qkv, W_proj, b_proj):
    from concourse.bass_utils import run_bass_kernel_spmd

    if "nc" not in _CACHE:
        _CACHE["nc"] = _build()
    nc = _CACHE["nc"]

    in_maps = _prep_inputs(x, W_qkv, W_proj, b_proj)
    res = run_bass_kernel_spmd(nc, in_maps, list(range(NUM_CORES)))
    return _unshard(res.results)
